# revision 24
# baseline (speedup 1.0000x reference)
"""DTLN-P2 stateful (2-layer LSTM separation net) Trainium2 Bass kernel.

Strategy: data-parallel over batch B=32 across 8 NeuronCores (4 per core),
no collectives. Per core:
  Phase A1 (upfront): encoder conv1x1 (fp16 PE matmuls) + instant-LayerNorm
        stats (ones-matmul column sums, Sqrt batched in one ACT table epoch).
  Phase A2 (interleaved): per-column mean/scale broadcast via K=1 matmuls,
        normalize encoder output -> fp16 [E, t, b] layout.
  Recurrence: the two LSTM layers run as two independent dependency chains
        (layer2 lags layer1 by LAG=33 steps) that overlap on the engines.
        Gate pre-activations (x@Wih + bias, LN folded into the weights, beta
        folded into the bias, g-gate pre-doubled) are materialized 25-step
        chunks at a time directly into PSUM banks by matmuls; the per-step
        h@Whh matmul (fp16, fast-weight-load) accumulates on top. Per step
        and layer: one fused sigmoid over all 4 gates (tanh(g) recovered as
        2*sigma(2g)-1 inside a scalar_tensor_tensor), 3 DVE ops for the cell
        update, one tanh, one output mul -> h (fp16) straight into the ys
        ring. State kept transposed [H=128 partitions, batch free]; cell
        state fp32.
  Phase C: mask dense + sigmoid + estimated = mask*enc + decoder conv1x1,
        pipelined chunk-by-chunk a fixed distance behind the layer-2 chain,
        PE work spread one op per step to keep the in-order PE queue clear.
"""
import os
import sys

sys.path.insert(0, "/opt/trn_rl_repo")

import numpy as np

B, F, T_FULL, E, H = 32, 512, 2000, 256, 128
EPS = 1e-7
NCORES = 8
BC = B // NCORES            # batch per core
S = 25                      # recurrence chunk (steps) held in one PSUM bank
LAG = S + 8                 # layer-2 step lag behind layer-1

# torch gate order (i, f, g, o) -> device gate slots [i, f, o, g]
_PERM = [0, 1, 3, 2]


def _reorder_gates(w):
    blocks = [w[i * H:(i + 1) * H] for i in range(4)]
    return np.concatenate([blocks[p] for p in _PERM], axis=0)


def host_prep(inputs):
    f32 = np.float32
    f16 = np.float16
    enc_w = np.asarray(inputs["enc_w"], f32)
    gamma = np.asarray(inputs["gamma"], f32)
    beta = np.asarray(inputs["beta"], f32)
    Wih1 = np.asarray(inputs["Wih1"], f32)
    Whh1 = np.asarray(inputs["Whh1"], f32)
    b1 = (np.asarray(inputs["bih1"], f32) + np.asarray(inputs["bhh1"], f32)
          + Wih1 @ beta)
    Wih2 = np.asarray(inputs["Wih2"], f32)
    Whh2 = np.asarray(inputs["Whh2"], f32)
    b2 = np.asarray(inputs["bih2"], f32) + np.asarray(inputs["bhh2"], f32)
    Wd = np.asarray(inputs["Wd"], f32)
    bd = np.asarray(inputs["bd"], f32)
    dec_w = np.asarray(inputs["dec_w"], f32)

    W1g = Wih1 * gamma[None, :]

    def g2(wt):
        # double the g-gate block (slot 3 after reorder): tanh(g)=2*sigma(2g)-1
        wt = wt.copy()
        wt[..., 3 * H:4 * H] *= 2.0
        return wt

    return {
        "ewt": np.ascontiguousarray(enc_w.T).astype(f16),           # [512,256]
        "w1s": np.ascontiguousarray(g2(_reorder_gates(W1g).T)).astype(f16),
        "b1r": np.ascontiguousarray(g2(_reorder_gates(b1[:, None]).T)).astype(f16),
        "whh1": np.ascontiguousarray(g2(_reorder_gates(Whh1).T)).astype(f16),
        "wih2": np.ascontiguousarray(g2(_reorder_gates(Wih2).T)).astype(f16),
        "b2r": np.ascontiguousarray(g2(_reorder_gates(b2[:, None]).T)).astype(f16),
        "whh2": np.ascontiguousarray(g2(_reorder_gates(Whh2).T)).astype(f16),
        "wdt": np.ascontiguousarray(Wd.T).astype(f16),              # [128,256]
        "bdr": np.ascontiguousarray(bd[None, :]).astype(f16),       # [1,256]
        "dwt": np.ascontiguousarray(dec_w.T).astype(f16),           # [256,512]
    }


def build_nc(T=T_FULL):
    import concourse.bass as bass
    import concourse.bacc as bacc
    import concourse.tile as tile
    from concourse import mybir

    f32 = mybir.dt.float32
    f16 = mybir.dt.float16
    AF = mybir.ActivationFunctionType
    OP = mybir.AluOpType

    assert T % S == 0
    NCH = T // S                       # number of recurrence chunks
    TW = min(250, T)                   # phase-A time window
    assert T % TW == 0
    WIN = min(2, NCH)                  # phase-C chunks per output DMA window
    assert NCH % WIN == 0
    NPAIR = T + LAG

    nc = bacc.Bacc()

    y1 = nc.dram_tensor("y1", [BC, F, T], f16, kind="ExternalInput")
    stin = nc.dram_tensor("stin", [H, 2, 2, BC], f32, kind="ExternalInput")
    ewt_d = nc.dram_tensor("ewt", [F, E], f16, kind="ExternalInput")
    w1s_d = nc.dram_tensor("w1s", [E, 4 * H], f16, kind="ExternalInput")
    b1r_d = nc.dram_tensor("b1r", [1, 4 * H], f16, kind="ExternalInput")
    whh1_d = nc.dram_tensor("whh1", [H, 4 * H], f16, kind="ExternalInput")
    wih2_d = nc.dram_tensor("wih2", [H, 4 * H], f16, kind="ExternalInput")
    b2r_d = nc.dram_tensor("b2r", [1, 4 * H], f16, kind="ExternalInput")
    whh2_d = nc.dram_tensor("whh2", [H, 4 * H], f16, kind="ExternalInput")
    wdt_d = nc.dram_tensor("wdt", [H, E], f16, kind="ExternalInput")
    bdr_d = nc.dram_tensor("bdr", [1, E], f16, kind="ExternalInput")
    dwt_d = nc.dram_tensor("dwt", [E, F], f16, kind="ExternalInput")
    dec_d = nc.dram_tensor("dec", [BC, F, T], f32, kind="ExternalOutput")
    stout_d = nc.dram_tensor("stout", [H, 2, 2, BC], f32, kind="ExternalOutput")

    with tile.TileContext(nc) as tc:
        with tc.tile_pool(name="consts", bufs=1) as consts, \
             tc.tile_pool(name="bigs", bufs=1) as bigs, \
             tc.tile_pool(name="y1p", bufs=2) as y1p, \
             tc.tile_pool(name="sqp", bufs=2) as sqp, \
             tc.tile_pool(name="lnp", bufs=2) as lnp, \
             tc.tile_pool(name="dtp", bufs=2) as dtp, \
             tc.tile_pool(name="sgp", bufs=4) as sgp, \
             tc.tile_pool(name="uvp", bufs=4) as uvp, \
             tc.tile_pool(name="mkp", bufs=2) as mkp, \
             tc.tile_pool(name="stg", bufs=2) as stgp, \
             tc.tile_pool(name="psp", bufs=1, space="PSUM") as psp:

            # ---------------- constant / persistent tiles ----------------
            ewt_sb = consts.tile([128, 4, E], f16, tag="ewt")
            nc.sync.dma_start(out=ewt_sb[:], in_=ewt_d.rearrange("(fb p) e -> p fb e", p=128))
            w1s_sb = consts.tile([128, 2, 4 * H], f16, tag="w1s")
            nc.sync.dma_start(out=w1s_sb[:], in_=w1s_d.rearrange("(eb p) m -> p eb m", p=128))
            dwt_sb = consts.tile([128, 2, F], f16, tag="dwt")
            nc.sync.dma_start(out=dwt_sb[:], in_=dwt_d.rearrange("(eb p) m -> p eb m", p=128))
            whh1_sb = consts.tile([128, 4 * H], f16, tag="whh1")
            nc.sync.dma_start(out=whh1_sb[:], in_=whh1_d[:])
            wih2_sb = consts.tile([128, 4 * H], f16, tag="wih2")
            nc.sync.dma_start(out=wih2_sb[:], in_=wih2_d[:])
            whh2_sb = consts.tile([128, 4 * H], f16, tag="whh2")
            nc.sync.dma_start(out=whh2_sb[:], in_=whh2_d[:])
            wdt_sb = consts.tile([128, E], f16, tag="wdt")
            nc.sync.dma_start(out=wdt_sb[:], in_=wdt_d[:])
            b1_sb = consts.tile([1, 4 * H], f16, tag="b1")
            nc.sync.dma_start(out=b1_sb[:], in_=b1r_d[:])
            b2_sb = consts.tile([1, 4 * H], f16, tag="b2")
            nc.sync.dma_start(out=b2_sb[:], in_=b2r_d[:])
            bdr_sb = consts.tile([1, E], f16, tag="bdr")
            nc.sync.dma_start(out=bdr_sb[:], in_=bdr_d[:])
            stin_sb = consts.tile([128, 2, 2, BC], f32, tag="stin")
            nc.sync.dma_start(out=stin_sb[:], in_=stin[:])
            ones_col = consts.tile([128, 1], f16, tag="onesc")
            nc.vector.memset(ones_col[:], 1.0)
            eps_sb = consts.tile([1, 1], f32, tag="eps")
            nc.vector.memset(eps_sb[:], EPS)
            ones16 = consts.tile([1, 128], f16, tag="ones16")
            nc.vector.memset(ones16[:], 1.0)

            enc_sb = bigs.tile([128, 2, BC, T], f16, tag="enc")       # [e, eb, b, t]
            encs_sb = bigs.tile([128, 2, T, BC], f16, tag="encs")     # [e, eb, t, b]
            ys_sb = bigs.tile([128, 2, T + 1, BC], f16, tag="ys")     # slot k = h(k-1)
            stout_sb = bigs.tile([128, 2, 2, BC], f32, tag="stout")
            c_t = bigs.tile([128, 2, BC], f32, tag="c")

            PS = psp.tile([128, 8, 512], f32, tag="ps")               # whole PSUM
            ps_ap = PS[:]           # for manual APs: ap[0] = partition entry

            def ps_view(off, dims):
                return bass.AP(tensor=ps_ap.tensor, offset=ps_ap.offset + off,
                               ap=[ps_ap.ap[0]] + dims)

            # -------- Phase A1 (upfront): encoder + LN stats (Sqrt batched) ----
            mean_sb = bigs.tile([1, BC, T], f16, tag="meanh")
            s_sb = bigs.tile([1, BC, T], f16, tag="sh")
            ach = 0
            for ic in range(T // TW):
                t0 = ic * TW
                for b in range(BC):
                    y1_t = y1p.tile([128, 4, TW], f16, tag="y1t")
                    nc.sync.dma_start(
                        out=y1_t[:],
                        in_=y1.rearrange("b (fb p) t -> b p fb t", p=128)[b, :, :, t0:t0 + TW])
                    for eb in range(2):
                        slot = ach % 2
                        ach += 1
                        for fb in range(4):
                            nc.tensor.matmul(
                                PS[:, slot, :TW],
                                ewt_sb[:, fb, eb * 128:(eb + 1) * 128],
                                y1_t[:, fb, :],
                                start=(fb == 0), stop=(fb == 3))
                        nc.scalar.copy(out=enc_sb[:, eb, b, t0:t0 + TW],
                                       in_=PS[:, slot, :TW])
                    # stats: column sums over e (256 = 2 partition blocks)
                    sb2 = 2 + (ach // 2) % 2            # alternate stats banks 2/3
                    for eb in range(2):
                        nc.tensor.matmul(PS[0:1, sb2, :TW], ones_col[:, 0:1],
                                         enc_sb[:, eb, b, t0:t0 + TW],
                                         start=(eb == 0), stop=False,
                                         skip_group_check=True)
                    for eb in range(2):
                        sq_t = sqp.tile([128, TW], f16, tag="sq")
                        nc.scalar.square(sq_t[:], enc_sb[:, eb, b, t0:t0 + TW])
                        nc.tensor.matmul(PS[0:1, sb2, TW:2 * TW], ones_col[:, 0:1],
                                         sq_t[:], start=False, stop=True,
                                         skip_group_check=True)
                    mean_t = lnp.tile([1, TW], f32, tag="mean")
                    nc.vector.tensor_scalar_mul(mean_t[:], PS[0:1, sb2, :TW], 1.0 / E)
                    m2_t = lnp.tile([1, TW], f32, tag="m2")
                    nc.vector.tensor_mul(m2_t[:], mean_t[:], mean_t[:])
                    var_t = lnp.tile([1, TW], f32, tag="var")
                    nc.vector.scalar_tensor_tensor(
                        out=var_t[:], in0=PS[0:1, sb2, TW:2 * TW], scalar=1.0 / E,
                        in1=m2_t[:], op0=OP.mult, op1=OP.subtract)
                    sd_t = lnp.tile([1, TW], f32, tag="m2", name="sd_t")
                    nc.scalar.activation(out=sd_t[:], in_=var_t[:], func=AF.Sqrt,
                                         bias=eps_sb[0:1, :], scale=1.0)
                    s_t = lnp.tile([1, TW], f32, tag="sinv")
                    nc.vector.reciprocal(out=s_t[:], in_=sd_t[:])
                    nc.vector.tensor_copy(mean_sb[0:1, b, t0:t0 + TW], mean_t[:])
                    nc.vector.tensor_copy(s_sb[0:1, b, t0:t0 + TW], s_t[:])

            # -------- Phase A2 (interleavable): broadcast + normalize -> fp16 --
            def phase_a2(ic):
                t0 = ic * TW
                for b in range(BC):
                    nc.tensor.matmul(PS[:, 6, :TW], ones16[0:1, 0:128],
                                     mean_sb[0:1, b, t0:t0 + TW], start=True, stop=True,
                                     skip_group_check=True)
                    nc.tensor.matmul(PS[:, 7, :TW], ones16[0:1, 0:128],
                                     s_sb[0:1, b, t0:t0 + TW], start=True, stop=True,
                                     skip_group_check=True)
                    for eb in range(2):
                        d_t = dtp.tile([128, TW], f32, tag="d")
                        nc.vector.tensor_sub(d_t[:], enc_sb[:, eb, b, t0:t0 + TW],
                                             PS[:, 6, :TW])
                        nc.vector.tensor_mul(
                            encs_sb[:, eb, t0:t0 + TW, b], d_t[:], PS[:, 7, :TW])

            phase_a2(0)

            # ---------------- recurrence prologue ----------------
            nc.vector.tensor_copy(c_t[:, :, :], stin_sb[:, :, 1, :])
            nc.vector.tensor_copy(ys_sb[:, :, 0, :], stin_sb[:, :, 0, :])

            ones_tb = ones16[0:1, :S * BC].rearrange("p (t b) -> p t b", b=BC)

            def fill_l1_gate(cc, g):
                # one start=True per chunk (clears the whole bank's has_written)
                bank = cc % 3
                out = PS[:, bank, g * (S * BC):(g + 1) * (S * BC)] \
                    .rearrange("p (t b) -> p t b", b=BC)
                for eb in range(2):
                    nc.tensor.matmul(
                        out, w1s_sb[:, eb, g * H:(g + 1) * H],
                        encs_sb[:, eb, cc * S:(cc + 1) * S, :],
                        start=(eb == 0 and g == 0), stop=False,
                        skip_group_check=True)
                nc.tensor.matmul(out, b1_sb[0:1, g * H:(g + 1) * H], ones_tb,
                                 start=False, stop=False, skip_group_check=True)

            def fill_l2_gate(cc, g):
                bank = 3 + cc % 3
                out = PS[:, bank, g * (S * BC):(g + 1) * (S * BC)] \
                    .rearrange("p (t b) -> p t b", b=BC)
                nc.tensor.matmul(out, wih2_sb[:, g * H:(g + 1) * H],
                                 ys_sb[:, 0, 1 + cc * S:1 + (cc + 1) * S, :],
                                 start=(g == 0), stop=False,
                                 skip_group_check=True)
                nc.tensor.matmul(out, b2_sb[0:1, g * H:(g + 1) * H], ones_tb,
                                 start=False, stop=False, skip_group_check=True)

            for g in range(4):
                fill_l1_gate(0, g)
            if NCH > 1:
                for g in range(4):
                    fill_l1_gate(1, g)

            # phase-C emission helper; step k in 0..6
            def phase_c_step(cc, k):
                mtile = mask_est[cc % 2]
                if k == 0:
                    for eb in range(2):
                        out = PS[:, 6, eb * (S * BC):(eb + 1) * (S * BC)] \
                            .rearrange("p (t b) -> p t b", b=BC)
                        nc.tensor.matmul(out, wdt_sb[:, eb * 128:(eb + 1) * 128],
                                         ys_sb[:, 1, 1 + cc * S:1 + (cc + 1) * S, :],
                                         start=(eb == 0), stop=False,
                                         skip_group_check=True)
                        nc.tensor.matmul(out, bdr_sb[0:1, eb * 128:(eb + 1) * 128],
                                         ones_tb, start=False, stop=(eb == 1),
                                         skip_group_check=True)
                elif k == 1:
                    nc.scalar.activation(
                        out=mtile[0][:], func=AF.Sigmoid,
                        in_=PS[:, 6, :2 * S * BC].rearrange(
                            "p (eb t b) -> p eb t b", eb=2, b=BC))
                    nc.vector.tensor_mul(
                        mtile[1][:], mtile[0][:],
                        enc_sb[:, :, :, cc * S:(cc + 1) * S]
                        .rearrange("p eb b t -> p eb t b"))
                elif k in (2, 3, 4, 5):
                    fb = k - 2
                    out = PS[:, 7, fb * (S * BC):(fb + 1) * (S * BC)] \
                        .rearrange("p (t b) -> p t b", b=BC)
                    for eb in range(2):
                        nc.tensor.matmul(out, dwt_sb[:, eb, fb * 128:(fb + 1) * 128],
                                         mtile[1][:, eb, :, :],
                                         start=(fb == 0 and eb == 0), stop=(fb == 3 and eb == 1),
                                         skip_group_check=True)
                else:  # k == 6: copy decoder psum into DMA staging
                    w = cc % WIN
                    nc.vector.tensor_copy(
                        stage[0][:, :, :, w * S:(w + 1) * S],
                        PS[:, 7, :4 * S * BC].rearrange(
                            "p (fb t b) -> p fb b t", fb=4, b=BC))
                    if w == WIN - 1:
                        w0 = (cc - WIN + 1) * S
                        for fb in range(4):
                            nc.sync.dma_start(
                                out=dec_d.rearrange("b (fb p) t -> p fb b t", p=128)
                                [:, fb, :, w0:w0 + WIN * S],
                                in_=stage[0][:, fb, :, :])

            mask_est = {}
            stage = {}

            # ---------------- main pair loop ----------------
            pend_c = []          # deferred phase-C work: (cc, k) queue
            for p in range(NPAIR):
                t1, t2 = p, p - LAG
                has1, has2 = t1 < T, t2 >= 0
                tl = p % S
                k1 = (t1 // S) % 3
                k2 = 3 + ((t2 // S) % 3 if has2 else 0)

                # --- interleaved phase A2 windows ---
                if p >= 5 and (p - 5) % TW == 0:
                    twn = (p - 5) // TW + 1
                    if twn < T // TW:
                        phase_a2(twn)

                # --- spread fills (one gate per pair) ---
                if has1 and tl < 4:
                    ccf = t1 // S + 2
                    if ccf < NCH:
                        fill_l1_gate(ccf, tl)
                tf = p - S - 1          # L2 fill pacing: chunk cc at p=cc*S+S+1..
                if tf >= 0 and tf % S < 4:
                    ccf = tf // S
                    if ccf < NCH:
                        fill_l2_gate(ccf, tf % S)

                # --- two per-layer chains, stage-interleaved so each engine's
                # FIFO alternates layers in natural firing order ---
                sg = sgp.tile([128, 2, 4, BC], f32, tag="sg")
                tc_tile = uvp.tile([128, 2, BC], f32, tag="tc")
                u_t = uvp.tile([128, 2, BC], f32, tag="u")
                v_t = uvp.tile([128, 2, BC], f32, tag="v")
                lays = [(l, t, kk) for l, t, kk in ((1, t2, k2), (0, t1, k1))
                        if (t1 < T if l == 0 else t2 >= 0)]
                for l, t, kk in lays:
                    tll = t % S
                    for g in range(4):
                        nc.tensor.matmul(
                            PS[:, kk, g * (S * BC) + tll * BC:
                               g * (S * BC) + (tll + 1) * BC],
                            (whh1_sb if l == 0 else whh2_sb)[:, g * H:(g + 1) * H],
                            ys_sb[:, l, t, :],
                            start=False, stop=True, skip_group_check=True)
                for l, t, kk in lays:
                    off = kk * 512 + (t % S) * BC
                    # all 4 gates in one sigmoid; slot3 holds sigma(2g)
                    nc.scalar.activation(
                        out=sg[:, l, 0:4, :], func=AF.Sigmoid,
                        in_=ps_view(off, [[S * BC, 4], [1, BC]]))
                for l, t, kk in lays:
                    # u' = (sigma(2g) - 0.5) * sigma(i)  [= i*tanh(g)/2]
                    nc.vector.scalar_tensor_tensor(
                        out=u_t[:, l, :], in0=sg[:, l, 3, :], scalar=0.5,
                        in1=sg[:, l, 0, :], op0=OP.subtract, op1=OP.mult)
                    nc.vector.tensor_mul(v_t[:, l, :], sg[:, l, 1, :], c_t[:, l, :])
                    # c = 2*u' + sigma(f)*c
                    nc.vector.scalar_tensor_tensor(
                        out=c_t[:, l, :], in0=u_t[:, l, :], scalar=2.0,
                        in1=v_t[:, l, :], op0=OP.mult, op1=OP.add)
                for l, t, kk in lays:
                    nc.scalar.activation(out=tc_tile[:, l, :], in_=c_t[:, l, :],
                                         func=AF.Tanh)
                for l, t, kk in lays:
                    nc.vector.tensor_mul(ys_sb[:, l, t + 1, :], sg[:, l, 2, :],
                                         tc_tile[:, l, :])

                # --- final-state capture (fp32) ---
                if t1 == T - 1:
                    nc.vector.tensor_mul(stout_sb[:, 0, 0, :], sg[:, 0, 2, :],
                                         tc_tile[:, 0, :])
                    nc.vector.tensor_copy(stout_sb[:, 0, 1, :], c_t[:, 0, :])
                if t2 == T - 1:
                    nc.vector.tensor_mul(stout_sb[:, 1, 0, :], sg[:, 1, 2, :],
                                         tc_tile[:, 1, :])
                    nc.vector.tensor_copy(stout_sb[:, 1, 1, :], c_t[:, 1, :])

                # --- phase C: enqueue after ys2 chunk completes, drain 1/pair ---
                if has2 and t2 % S == S - 1:
                    cc = t2 // S
                    mask_est[cc % 2] = (
                        mkp.tile([128, 2, S, BC], f16, tag="mask", name="mask"),
                        mkp.tile([128, 2, S, BC], f16, tag="est", name="est"))
                    if cc % WIN == 0:
                        stage[0] = stgp.tile([128, 4, BC, WIN * S], f32,
                                             tag="stage", name="stage")
                    for k in range(7):
                        pend_c.append((cc, k))
                if pend_c:
                    cc, k = pend_c.pop(0)
                    phase_c_step(cc, k)

            # drain remaining phase-C work
            while pend_c:
                cc, k = pend_c.pop(0)
                phase_c_step(cc, k)

            nc.sync.dma_start(out=stout_d[:], in_=stout_sb[:])

    nc.compile()
    return nc


_CACHE = {}


def _get_nc(T):
    if T not in _CACHE:
        _CACHE[T] = build_nc(T)
    return _CACHE[T]


def kernel(**inputs):
    from concourse.bass_utils import run_bass_kernel_spmd

    y1 = np.asarray(inputs["y1"], np.float16)
    in_state2 = np.asarray(inputs["in_state2"], np.float32)
    T = y1.shape[2]
    nc = _get_nc(T)
    W = host_prep(inputs)

    in_maps = []
    for c in range(NCORES):
        bs = slice(c * BC, (c + 1) * BC)
        stc = np.ascontiguousarray(
            np.transpose(in_state2[:, bs], (2, 0, 3, 1)))   # [H, l, hc, b]
        m = {"y1": np.ascontiguousarray(y1[bs]), "stin": stc}
        m.update(W)
        in_maps.append(m)

    res = run_bass_kernel_spmd(nc, in_maps, core_ids=list(range(NCORES)))

    decoded = np.concatenate([res.results[c]["dec"] for c in range(NCORES)], axis=0)
    out_state = np.zeros((2, B, H, 2), np.float32)
    for c in range(NCORES):
        st = res.results[c]["stout"]                        # [H, l, hc, b]
        out_state[:, c * BC:(c + 1) * BC] = np.transpose(st, (1, 3, 0, 2))
    return decoded, out_state


# revision 27
# speedup vs baseline: 1.0032x; 1.0032x over previous
"""DTLN-P2 stateful (2-layer LSTM separation net) Trainium2 Bass kernel.

Strategy: data-parallel over batch B=32 across 8 NeuronCores (4 per core),
no collectives. Per core:
  Phase A1 (upfront): encoder conv1x1 (fp16 PE matmuls) + instant-LayerNorm
        stats (ones-matmul column sums, Sqrt batched in one ACT table epoch).
  Phase A2 (interleaved): per-column mean/scale broadcast via K=1 matmuls,
        normalize encoder output -> fp16 [E, t, b] layout.
  Recurrence: the two LSTM layers run as two independent dependency chains
        (layer2 lags layer1 by LAG=33 steps) that overlap on the engines.
        Gate pre-activations (x@Wih + bias, LN folded into the weights, beta
        folded into the bias, g-gate pre-doubled) are materialized 25-step
        chunks at a time directly into PSUM banks by matmuls; the per-step
        h@Whh matmul (fp16, fast-weight-load) accumulates on top. Per step
        and layer: one fused sigmoid over all 4 gates (tanh(g) recovered as
        2*sigma(2g)-1 inside a scalar_tensor_tensor), 3 DVE ops for the cell
        update, one tanh, one output mul -> h (fp16) straight into the ys
        ring. State kept transposed [H=128 partitions, batch free]; cell
        state fp32.
  Phase C: mask dense + sigmoid + estimated = mask*enc + decoder conv1x1,
        pipelined chunk-by-chunk a fixed distance behind the layer-2 chain,
        PE work spread one op per step to keep the in-order PE queue clear.
"""
import os
import sys

sys.path.insert(0, "/opt/trn_rl_repo")

import numpy as np

B, F, T_FULL, E, H = 32, 512, 2000, 256, 128
EPS = 1e-7
NCORES = 8
BC = B // NCORES            # batch per core
S = 25                      # recurrence chunk (steps) held in one PSUM bank
LAG = S + 8                 # layer-2 step lag behind layer-1

# torch gate order (i, f, g, o) -> device gate slots [i, f, o, g]
_PERM = [0, 1, 3, 2]


def _reorder_gates(w):
    blocks = [w[i * H:(i + 1) * H] for i in range(4)]
    return np.concatenate([blocks[p] for p in _PERM], axis=0)


def host_prep(inputs):
    f32 = np.float32
    f16 = np.float16
    enc_w = np.asarray(inputs["enc_w"], f32)
    gamma = np.asarray(inputs["gamma"], f32)
    beta = np.asarray(inputs["beta"], f32)
    Wih1 = np.asarray(inputs["Wih1"], f32)
    Whh1 = np.asarray(inputs["Whh1"], f32)
    b1 = (np.asarray(inputs["bih1"], f32) + np.asarray(inputs["bhh1"], f32)
          + Wih1 @ beta)
    Wih2 = np.asarray(inputs["Wih2"], f32)
    Whh2 = np.asarray(inputs["Whh2"], f32)
    b2 = np.asarray(inputs["bih2"], f32) + np.asarray(inputs["bhh2"], f32)
    Wd = np.asarray(inputs["Wd"], f32)
    bd = np.asarray(inputs["bd"], f32)
    dec_w = np.asarray(inputs["dec_w"], f32)

    W1g = Wih1 * gamma[None, :]

    def g2(wt):
        # double the g-gate block (slot 3 after reorder): tanh(g)=2*sigma(2g)-1
        wt = wt.copy()
        wt[..., 3 * H:4 * H] *= 2.0
        return wt

    return {
        "ewt": np.ascontiguousarray(enc_w.T).astype(f16),           # [512,256]
        "w1s": np.ascontiguousarray(g2(_reorder_gates(W1g).T)).astype(f16),
        "b1r": np.ascontiguousarray(g2(_reorder_gates(b1[:, None]).T)).astype(f16),
        "whh1": np.ascontiguousarray(g2(_reorder_gates(Whh1).T)).astype(f16),
        "wih2": np.ascontiguousarray(g2(_reorder_gates(Wih2).T)).astype(f16),
        "b2r": np.ascontiguousarray(g2(_reorder_gates(b2[:, None]).T)).astype(f16),
        "whh2": np.ascontiguousarray(g2(_reorder_gates(Whh2).T)).astype(f16),
        "wdt": np.ascontiguousarray(Wd.T).astype(f16),              # [128,256]
        "bdr": np.ascontiguousarray(bd[None, :]).astype(f16),       # [1,256]
        "dwt": np.ascontiguousarray(dec_w.T).astype(f16),           # [256,512]
    }


def build_nc(T=T_FULL):
    import concourse.bass as bass
    import concourse.bacc as bacc
    import concourse.tile as tile
    from concourse import mybir

    f32 = mybir.dt.float32
    f16 = mybir.dt.float16
    AF = mybir.ActivationFunctionType
    OP = mybir.AluOpType

    assert T % S == 0
    NCH = T // S                       # number of recurrence chunks
    TW = min(250, T)                   # phase-A time window
    assert T % TW == 0
    WIN = min(2, NCH)                  # phase-C chunks per output DMA window
    assert NCH % WIN == 0
    NPAIR = T + LAG

    nc = bacc.Bacc()

    y1 = nc.dram_tensor("y1", [BC, F, T], f16, kind="ExternalInput")
    stin = nc.dram_tensor("stin", [H, 2, 2, BC], f32, kind="ExternalInput")
    ewt_d = nc.dram_tensor("ewt", [F, E], f16, kind="ExternalInput")
    w1s_d = nc.dram_tensor("w1s", [E, 4 * H], f16, kind="ExternalInput")
    b1r_d = nc.dram_tensor("b1r", [1, 4 * H], f16, kind="ExternalInput")
    whh1_d = nc.dram_tensor("whh1", [H, 4 * H], f16, kind="ExternalInput")
    wih2_d = nc.dram_tensor("wih2", [H, 4 * H], f16, kind="ExternalInput")
    b2r_d = nc.dram_tensor("b2r", [1, 4 * H], f16, kind="ExternalInput")
    whh2_d = nc.dram_tensor("whh2", [H, 4 * H], f16, kind="ExternalInput")
    wdt_d = nc.dram_tensor("wdt", [H, E], f16, kind="ExternalInput")
    bdr_d = nc.dram_tensor("bdr", [1, E], f16, kind="ExternalInput")
    dwt_d = nc.dram_tensor("dwt", [E, F], f16, kind="ExternalInput")
    dec_d = nc.dram_tensor("dec", [BC, F, T], f32, kind="ExternalOutput")
    stout_d = nc.dram_tensor("stout", [H, 2, 2, BC], f32, kind="ExternalOutput")

    with tile.TileContext(nc) as tc:
        with tc.tile_pool(name="consts", bufs=1) as consts, \
             tc.tile_pool(name="bigs", bufs=1) as bigs, \
             tc.tile_pool(name="y1p", bufs=2) as y1p, \
             tc.tile_pool(name="sqp", bufs=2) as sqp, \
             tc.tile_pool(name="lnp", bufs=2) as lnp, \
             tc.tile_pool(name="dtp", bufs=2) as dtp, \
             tc.tile_pool(name="sgp", bufs=4) as sgp, \
             tc.tile_pool(name="uvp", bufs=4) as uvp, \
             tc.tile_pool(name="mkp", bufs=2) as mkp, \
             tc.tile_pool(name="stg", bufs=2) as stgp, \
             tc.tile_pool(name="psp", bufs=1, space="PSUM") as psp:

            # ---------------- constant / persistent tiles ----------------
            ewt_sb = consts.tile([128, 4, E], f16, tag="ewt")
            nc.sync.dma_start(out=ewt_sb[:], in_=ewt_d.rearrange("(fb p) e -> p fb e", p=128))
            w1s_sb = consts.tile([128, 2, 4 * H], f16, tag="w1s")
            nc.sync.dma_start(out=w1s_sb[:], in_=w1s_d.rearrange("(eb p) m -> p eb m", p=128))
            dwt_sb = consts.tile([128, 2, F], f16, tag="dwt")
            nc.sync.dma_start(out=dwt_sb[:], in_=dwt_d.rearrange("(eb p) m -> p eb m", p=128))
            whh1_sb = consts.tile([128, 4 * H], f16, tag="whh1")
            nc.sync.dma_start(out=whh1_sb[:], in_=whh1_d[:])
            wih2_sb = consts.tile([128, 4 * H], f16, tag="wih2")
            nc.sync.dma_start(out=wih2_sb[:], in_=wih2_d[:])
            whh2_sb = consts.tile([128, 4 * H], f16, tag="whh2")
            nc.sync.dma_start(out=whh2_sb[:], in_=whh2_d[:])
            wdt_sb = consts.tile([128, E], f16, tag="wdt")
            nc.sync.dma_start(out=wdt_sb[:], in_=wdt_d[:])
            b1_sb = consts.tile([1, 4 * H], f16, tag="b1")
            nc.sync.dma_start(out=b1_sb[:], in_=b1r_d[:])
            b2_sb = consts.tile([1, 4 * H], f16, tag="b2")
            nc.sync.dma_start(out=b2_sb[:], in_=b2r_d[:])
            bdr_sb = consts.tile([1, E], f16, tag="bdr")
            nc.sync.dma_start(out=bdr_sb[:], in_=bdr_d[:])
            stin_sb = consts.tile([128, 2, 2, BC], f32, tag="stin")
            nc.sync.dma_start(out=stin_sb[:], in_=stin[:])
            ones_col = consts.tile([128, 1], f16, tag="onesc")
            nc.vector.memset(ones_col[:], 1.0)
            eps_sb = consts.tile([1, 1], f32, tag="eps")
            nc.vector.memset(eps_sb[:], EPS)
            ones16 = consts.tile([1, 128], f16, tag="ones16")
            nc.vector.memset(ones16[:], 1.0)

            enc_sb = bigs.tile([128, 2, BC, T], f16, tag="enc")       # [e, eb, b, t]
            encs_sb = bigs.tile([128, 2, T, BC], f16, tag="encs")     # [e, eb, t, b]
            ys_sb = bigs.tile([128, 2, T + 1, BC], f16, tag="ys")     # slot k = h(k-1)
            stout_sb = bigs.tile([128, 2, 2, BC], f32, tag="stout")
            c_t = bigs.tile([128, 2, BC], f32, tag="c")

            PS = psp.tile([128, 8, 512], f32, tag="ps")               # whole PSUM
            ps_ap = PS[:]           # for manual APs: ap[0] = partition entry

            def ps_view(off, dims):
                return bass.AP(tensor=ps_ap.tensor, offset=ps_ap.offset + off,
                               ap=[ps_ap.ap[0]] + dims)

            # -------- Phase A1 (upfront): encoder + LN stats (Sqrt batched) ----
            mean_sb = bigs.tile([1, BC, T], f16, tag="meanh")
            s_sb = bigs.tile([1, BC, T], f16, tag="sh")
            ach = 0
            for ic in range(T // TW):
                t0 = ic * TW
                for b in range(BC):
                    y1_t = y1p.tile([128, 4, TW], f16, tag="y1t")
                    nc.sync.dma_start(
                        out=y1_t[:],
                        in_=y1.rearrange("b (fb p) t -> b p fb t", p=128)[b, :, :, t0:t0 + TW])
                    for eb in range(2):
                        slot = ach % 2
                        ach += 1
                        for fb in range(4):
                            nc.tensor.matmul(
                                PS[:, slot, :TW],
                                ewt_sb[:, fb, eb * 128:(eb + 1) * 128],
                                y1_t[:, fb, :],
                                start=(fb == 0), stop=(fb == 3))
                        nc.scalar.copy(out=enc_sb[:, eb, b, t0:t0 + TW],
                                       in_=PS[:, slot, :TW])
                    # stats: column sums over e (256 = 2 partition blocks)
                    sb2 = 2 + (ach // 2) % 2            # alternate stats banks 2/3
                    for eb in range(2):
                        nc.tensor.matmul(PS[0:1, sb2, :TW], ones_col[:, 0:1],
                                         enc_sb[:, eb, b, t0:t0 + TW],
                                         start=(eb == 0), stop=False,
                                         skip_group_check=True)
                    for eb in range(2):
                        sq_t = sqp.tile([128, TW], f16, tag="sq")
                        nc.gpsimd.tensor_mul(sq_t[:], enc_sb[:, eb, b, t0:t0 + TW],
                                             enc_sb[:, eb, b, t0:t0 + TW])
                        nc.tensor.matmul(PS[0:1, sb2, TW:2 * TW], ones_col[:, 0:1],
                                         sq_t[:], start=False, stop=True,
                                         skip_group_check=True)
                    mean_v = mean_sb[0:1, b, t0:t0 + TW]
                    with nc.allow_low_precision(reason="LN mean/scale stored fp16"):
                        nc.vector.tensor_scalar_mul(mean_v, PS[0:1, sb2, :TW], 1.0 / E)
                    m2_t = lnp.tile([1, TW], f32, tag="m2")
                    nc.vector.tensor_mul(m2_t[:], mean_v, mean_v)
                    var_t = lnp.tile([1, TW], f32, tag="var")
                    nc.vector.scalar_tensor_tensor(
                        out=var_t[:], in0=PS[0:1, sb2, TW:2 * TW], scalar=1.0 / E,
                        in1=m2_t[:], op0=OP.mult, op1=OP.subtract)
                    sd_t = lnp.tile([1, TW], f32, tag="m2", name="sd_t")
                    nc.scalar.activation(out=sd_t[:], in_=var_t[:], func=AF.Sqrt,
                                         bias=eps_sb[0:1, :], scale=1.0)
                    with nc.allow_low_precision(reason="LN scale stored fp16"):
                        nc.vector.reciprocal(out=s_sb[0:1, b, t0:t0 + TW],
                                             in_=sd_t[:])

            # -------- Phase A2 (interleavable): broadcast + normalize -> fp16 --
            def phase_a2(ic):
                t0 = ic * TW
                for b in range(BC):
                    nc.tensor.matmul(PS[:, 6, :TW], ones16[0:1, 0:128],
                                     mean_sb[0:1, b, t0:t0 + TW], start=True, stop=True,
                                     skip_group_check=True)
                    nc.tensor.matmul(PS[:, 7, :TW], ones16[0:1, 0:128],
                                     s_sb[0:1, b, t0:t0 + TW], start=True, stop=True,
                                     skip_group_check=True)
                    for eb in range(2):
                        d_t = dtp.tile([128, TW], f32, tag="d")
                        nc.vector.tensor_sub(d_t[:], enc_sb[:, eb, b, t0:t0 + TW],
                                             PS[:, 6, :TW])
                        nc.vector.tensor_mul(
                            encs_sb[:, eb, t0:t0 + TW, b], d_t[:], PS[:, 7, :TW])

            phase_a2(0)

            # ---------------- recurrence prologue ----------------
            nc.vector.tensor_copy(c_t[:, :, :], stin_sb[:, :, 1, :])
            nc.vector.tensor_copy(ys_sb[:, :, 0, :], stin_sb[:, :, 0, :])

            ones_tb = ones16[0:1, :S * BC].rearrange("p (t b) -> p t b", b=BC)

            def fill_l1_gate(cc, g):
                # one start=True per chunk (clears the whole bank's has_written)
                bank = cc % 3
                out = PS[:, bank, g * (S * BC):(g + 1) * (S * BC)] \
                    .rearrange("p (t b) -> p t b", b=BC)
                for eb in range(2):
                    nc.tensor.matmul(
                        out, w1s_sb[:, eb, g * H:(g + 1) * H],
                        encs_sb[:, eb, cc * S:(cc + 1) * S, :],
                        start=(eb == 0 and g == 0), stop=False,
                        skip_group_check=True)
                nc.tensor.matmul(out, b1_sb[0:1, g * H:(g + 1) * H], ones_tb,
                                 start=False, stop=False, skip_group_check=True)

            def fill_l2_gate(cc, g):
                bank = 3 + cc % 3
                out = PS[:, bank, g * (S * BC):(g + 1) * (S * BC)] \
                    .rearrange("p (t b) -> p t b", b=BC)
                nc.tensor.matmul(out, wih2_sb[:, g * H:(g + 1) * H],
                                 ys_sb[:, 0, 1 + cc * S:1 + (cc + 1) * S, :],
                                 start=(g == 0), stop=False,
                                 skip_group_check=True)
                nc.tensor.matmul(out, b2_sb[0:1, g * H:(g + 1) * H], ones_tb,
                                 start=False, stop=False, skip_group_check=True)

            for g in range(4):
                fill_l1_gate(0, g)
            if NCH > 1:
                for g in range(4):
                    fill_l1_gate(1, g)

            # phase-C emission helper; step k in 0..6
            def phase_c_step(cc, k):
                mtile = mask_est[cc % 2]
                if k == 0:
                    for eb in range(2):
                        out = PS[:, 6, eb * (S * BC):(eb + 1) * (S * BC)] \
                            .rearrange("p (t b) -> p t b", b=BC)
                        nc.tensor.matmul(out, wdt_sb[:, eb * 128:(eb + 1) * 128],
                                         ys_sb[:, 1, 1 + cc * S:1 + (cc + 1) * S, :],
                                         start=(eb == 0), stop=False,
                                         skip_group_check=True)
                        nc.tensor.matmul(out, bdr_sb[0:1, eb * 128:(eb + 1) * 128],
                                         ones_tb, start=False, stop=(eb == 1),
                                         skip_group_check=True)
                elif k == 1:
                    nc.scalar.activation(
                        out=mtile[0][:], func=AF.Sigmoid,
                        in_=PS[:, 6, :2 * S * BC].rearrange(
                            "p (eb t b) -> p eb t b", eb=2, b=BC))
                    nc.vector.tensor_mul(
                        mtile[1][:], mtile[0][:],
                        enc_sb[:, :, :, cc * S:(cc + 1) * S]
                        .rearrange("p eb b t -> p eb t b"))
                elif k in (2, 3, 4, 5):
                    fb = k - 2
                    out = PS[:, 7, fb * (S * BC):(fb + 1) * (S * BC)] \
                        .rearrange("p (t b) -> p t b", b=BC)
                    for eb in range(2):
                        nc.tensor.matmul(out, dwt_sb[:, eb, fb * 128:(fb + 1) * 128],
                                         mtile[1][:, eb, :, :],
                                         start=(fb == 0 and eb == 0), stop=(fb == 3 and eb == 1),
                                         skip_group_check=True)
                else:  # k == 6: copy decoder psum into DMA staging
                    w = cc % WIN
                    nc.vector.tensor_copy(
                        stage[0][:, :, :, w * S:(w + 1) * S],
                        PS[:, 7, :4 * S * BC].rearrange(
                            "p (fb t b) -> p fb b t", fb=4, b=BC))
                    if w == WIN - 1:
                        w0 = (cc - WIN + 1) * S
                        for fb in range(4):
                            nc.sync.dma_start(
                                out=dec_d.rearrange("b (fb p) t -> p fb b t", p=128)
                                [:, fb, :, w0:w0 + WIN * S],
                                in_=stage[0][:, fb, :, :])

            mask_est = {}
            stage = {}

            # ---------------- main pair loop ----------------
            pend_c = []          # deferred phase-C work: (cc, k) queue
            for p in range(NPAIR):
                t1, t2 = p, p - LAG
                has1, has2 = t1 < T, t2 >= 0
                tl = p % S
                k1 = (t1 // S) % 3
                k2 = 3 + ((t2 // S) % 3 if has2 else 0)

                # --- interleaved phase A2 windows ---
                if p >= 5 and (p - 5) % TW == 0:
                    twn = (p - 5) // TW + 1
                    if twn < T // TW:
                        phase_a2(twn)

                # --- spread fills (one gate per pair) ---
                if has1 and tl < 4:
                    ccf = t1 // S + 2
                    if ccf < NCH:
                        fill_l1_gate(ccf, tl)
                tf = p - S - 1          # L2 fill pacing: chunk cc at p=cc*S+S+1..
                if tf >= 0 and tf % S < 4:
                    ccf = tf // S
                    if ccf < NCH:
                        fill_l2_gate(ccf, tf % S)

                # --- two per-layer chains, stage-interleaved so each engine's
                # FIFO alternates layers in natural firing order ---
                sg = sgp.tile([128, 2, 4, BC], f32, tag="sg")
                tc_tile = uvp.tile([128, 2, BC], f32, tag="tc")
                u_t = uvp.tile([128, 2, BC], f32, tag="u")
                v_t = uvp.tile([128, 2, BC], f32, tag="v")
                lays = [(l, t, kk) for l, t, kk in ((1, t2, k2), (0, t1, k1))
                        if (t1 < T if l == 0 else t2 >= 0)]
                for l, t, kk in lays:
                    tll = t % S
                    for g in range(4):
                        nc.tensor.matmul(
                            PS[:, kk, g * (S * BC) + tll * BC:
                               g * (S * BC) + (tll + 1) * BC],
                            (whh1_sb if l == 0 else whh2_sb)[:, g * H:(g + 1) * H],
                            ys_sb[:, l, t, :],
                            start=False, stop=True, skip_group_check=True)
                for l, t, kk in lays:
                    off = kk * 512 + (t % S) * BC
                    # all 4 gates in one sigmoid; slot3 holds sigma(2g)
                    nc.scalar.activation(
                        out=sg[:, l, 0:4, :], func=AF.Sigmoid,
                        in_=ps_view(off, [[S * BC, 4], [1, BC]]))
                for l, t, kk in lays:
                    # u' = (sigma(2g) - 0.5) * sigma(i)  [= i*tanh(g)/2]
                    nc.vector.scalar_tensor_tensor(
                        out=u_t[:, l, :], in0=sg[:, l, 3, :], scalar=0.5,
                        in1=sg[:, l, 0, :], op0=OP.subtract, op1=OP.mult)
                    nc.vector.tensor_mul(v_t[:, l, :], sg[:, l, 1, :], c_t[:, l, :])
                    # c = 2*u' + sigma(f)*c
                    nc.vector.scalar_tensor_tensor(
                        out=c_t[:, l, :], in0=u_t[:, l, :], scalar=2.0,
                        in1=v_t[:, l, :], op0=OP.mult, op1=OP.add)
                for l, t, kk in lays:
                    nc.scalar.activation(out=tc_tile[:, l, :], in_=c_t[:, l, :],
                                         func=AF.Tanh)
                for l, t, kk in lays:
                    nc.vector.tensor_mul(ys_sb[:, l, t + 1, :], sg[:, l, 2, :],
                                         tc_tile[:, l, :])

                # --- final-state capture (fp32) ---
                if t1 == T - 1:
                    nc.vector.tensor_mul(stout_sb[:, 0, 0, :], sg[:, 0, 2, :],
                                         tc_tile[:, 0, :])
                    nc.vector.tensor_copy(stout_sb[:, 0, 1, :], c_t[:, 0, :])
                if t2 == T - 1:
                    nc.vector.tensor_mul(stout_sb[:, 1, 0, :], sg[:, 1, 2, :],
                                         tc_tile[:, 1, :])
                    nc.vector.tensor_copy(stout_sb[:, 1, 1, :], c_t[:, 1, :])

                # --- phase C: enqueue after ys2 chunk completes, drain 1/pair ---
                if has2 and t2 % S == S - 1:
                    cc = t2 // S
                    mask_est[cc % 2] = (
                        mkp.tile([128, 2, S, BC], f16, tag="mask", name="mask"),
                        mkp.tile([128, 2, S, BC], f16, tag="est", name="est"))
                    if cc % WIN == 0:
                        stage[0] = stgp.tile([128, 4, BC, WIN * S], f32,
                                             tag="stage", name="stage")
                    for k in range(7):
                        pend_c.append((cc, k))
                if pend_c:
                    cc, k = pend_c.pop(0)
                    phase_c_step(cc, k)

            # drain remaining phase-C work
            while pend_c:
                cc, k = pend_c.pop(0)
                phase_c_step(cc, k)

            nc.sync.dma_start(out=stout_d[:], in_=stout_sb[:])

    nc.compile()
    return nc


_CACHE = {}


def _get_nc(T):
    if T not in _CACHE:
        _CACHE[T] = build_nc(T)
    return _CACHE[T]


def kernel(**inputs):
    from concourse.bass_utils import run_bass_kernel_spmd

    y1 = np.asarray(inputs["y1"], np.float16)
    in_state2 = np.asarray(inputs["in_state2"], np.float32)
    T = y1.shape[2]
    nc = _get_nc(T)
    W = host_prep(inputs)

    in_maps = []
    for c in range(NCORES):
        bs = slice(c * BC, (c + 1) * BC)
        stc = np.ascontiguousarray(
            np.transpose(in_state2[:, bs], (2, 0, 3, 1)))   # [H, l, hc, b]
        m = {"y1": np.ascontiguousarray(y1[bs]), "stin": stc}
        m.update(W)
        in_maps.append(m)

    res = run_bass_kernel_spmd(nc, in_maps, core_ids=list(range(NCORES)))

    decoded = np.concatenate([res.results[c]["dec"] for c in range(NCORES)], axis=0)
    out_state = np.zeros((2, B, H, 2), np.float32)
    for c in range(NCORES):
        st = res.results[c]["stout"]                        # [H, l, hc, b]
        out_state[:, c * BC:(c + 1) * BC] = np.transpose(st, (1, 3, 0, 2))
    return decoded, out_state
